# revision 25
# baseline (speedup 1.0000x reference)
"""DiT block kernel for 8 Trainium2 NeuronCores (Bass/Tile).

Sharding: sequence dim L=4096 split 8 ways (512 query rows per core).
Each core computes K/V for the full sequence (replicated; avoids any
cross-core collective) but only its own 512 queries through attention,
the out-projection, and the FFN. Inputs are rotated host-side so every
core's local rows sit at positions [0, 512) -> one SPMD program.
Weights are cast to bf16 host-side; accumulation is fp32 in PSUM;
layernorm statistics and residuals are fp32.
"""

import sys

sys.path.insert(0, "/opt/trn_rl_repo")

from contextlib import ExitStack

import numpy as np
import ml_dtypes

import concourse.bass as bass
import concourse.bacc as bacc
import concourse.tile as tile
import concourse.mybir as mybir
from concourse.bass_utils import run_bass_kernel_spmd
from concourse.masks import make_identity

F32 = mybir.dt.float32
BF16 = mybir.dt.bfloat16
AF = mybir.ActivationFunctionType
OP = mybir.AluOpType

L, D, H, HD, DM = 4096, 768, 12, 64, 3072
NCORES = 8
LQ = L // NCORES  # 512 local query rows
P = 128
EPS = 1e-5
NLC = L // 512  # 8 l-chunks of 512
NKC = L // P  # 32 k-chunks of 128
NQC = LQ // P  # 4 local q-chunks of 128
NDC = D // P  # 6 chunks of the model dim
NHP = H // 2  # 6 head pairs
NMC = DM // P  # 24 chunks of the FFN hidden dim


def _declare_params(nc):
    dp = nc.declare_dram_parameter
    t = {}
    t["x"] = dp("x", [L, D], F32, isOutput=False)
    t["cond_t"] = dp("cond_t", [P, NDC], F32, isOutput=False)
    t["w_adaln1"] = dp("w_adaln1", [D, 3 * D], BF16, isOutput=False)
    t["w_adaln2"] = dp("w_adaln2", [D, 3 * D], BF16, isOutput=False)
    t["b_adaln1_col"] = dp("b_adaln1_col", [P, 12], F32, isOutput=False)
    t["b_adaln2_col"] = dp("b_adaln2_col", [P, 12], F32, isOutput=False)
    t["b_adaln1_gate"] = dp("b_adaln1_gate", [1, D], F32, isOutput=False)
    t["b_adaln2_gate"] = dp("b_adaln2_gate", [1, D], F32, isOutput=False)
    t["w_qkv"] = dp("w_qkv", [D, 3 * D], BF16, isOutput=False)
    t["b_qkv_col"] = dp("b_qkv_col", [P, 18], F32, isOutput=False)
    t["b_v_b"] = dp("b_v_b", [P, D], F32, isOutput=False)
    t["w_attn_out"] = dp("w_attn_out", [D, D], BF16, isOutput=False)
    t["b_attn_b"] = dp("b_attn_b", [P, D], F32, isOutput=False)
    t["w_ffn1"] = dp("w_ffn1", [D, DM], BF16, isOutput=False)
    t["b_ffn1_col"] = dp("b_ffn1_col", [P, NMC], F32, isOutput=False)
    t["w_ffn2"] = dp("w_ffn2", [DM, D], BF16, isOutput=False)
    t["b_ffn2_b"] = dp("b_ffn2_b", [P, D], F32, isOutput=False)
    t["out"] = dp("out", [LQ, D], F32, isOutput=True)
    return t


def _build_body(nc, tc, ctx, t):
    mm = nc.tensor.matmul
    dma = nc.sync.dma_start
    v = nc.vector
    act = nc.scalar.activation

    const = ctx.enter_context(tc.tile_pool(name="const", bufs=1))
    identity = const.tile([P, P], BF16)
    make_identity(nc, identity)
    eps_t = const.tile([P, 1], F32)
    v.memset(eps_t, EPS)

    # ---------------- phase A: cond path (SiLU + AdaLN projections) -------
    adaln = ctx.enter_context(tc.tile_pool(name="adaln", bufs=1))
    sc_bf = adaln.tile([P, NDC], BF16)
    # shift/scale in column-transposed layout; (1+scale) precomputed
    sh1_col = adaln.tile([P, NDC], F32)
    sp1_col = adaln.tile([P, NDC], F32)
    sh2_col = adaln.tile([P, NDC], F32)
    sp2_col = adaln.tile([P, NDC], F32)
    g1_b = adaln.tile([P, D], F32)
    g2_b = adaln.tile([P, D], F32)

    with ExitStack() as phA:
        pool = phA.enter_context(tc.tile_pool(name="phA", bufs=1))
        dram_pool = phA.enter_context(tc.tile_pool(name="phAdram", bufs=1, space="DRAM"))
        psA1 = phA.enter_context(tc.tile_pool(name="psA1", bufs=2, space="PSUM"))
        psA2 = phA.enter_context(tc.tile_pool(name="psA2", bufs=2, space="PSUM"))

        cond_sb = pool.tile([P, NDC], F32)
        dma(out=cond_sb[:], in_=t["cond_t"][:])
        act(sc_bf[:], cond_sb[:], AF.Silu)

        wa1 = pool.tile([P, NDC, 3 * D], BF16)
        dma(out=wa1[:], in_=t["w_adaln1"].rearrange("(c p) m -> p c m", p=P))
        wa2 = pool.tile([P, NDC, 3 * D], BF16)
        dma(out=wa2[:], in_=t["w_adaln2"].rearrange("(c p) m -> p c m", p=P))
        b1c = pool.tile([P, 12], F32)
        dma(out=b1c[:], in_=t["b_adaln1_col"][:])
        b2c = pool.tile([P, 12], F32)
        dma(out=b2c[:], in_=t["b_adaln2_col"][:])
        b1g = pool.tile([1, D], F32)
        dma(out=b1g[:], in_=t["b_adaln1_gate"][:])
        b2g = pool.tile([1, D], F32)
        dma(out=b2g[:], in_=t["b_adaln2_gate"][:])

        for r, (wa, bc, bg, sh_col, sp_col, g_b) in enumerate(
            [
                (wa1, b1c, b1g, sh1_col, sp1_col, g1_b),
                (wa2, b2c, b2g, sh2_col, sp2_col, g2_b),
            ]
        ):
            # shift+scale in row layout (3 M=1 matmuls), then one
            # SBUF->SBUF rearrange DMA into the column-transposed layout
            arow = pool.tile([1, 12 * P], F32, name=f"arow{r}")
            for j in range(3):
                ps = psA1.tile([1, 512], F32)
                for dc in range(NDC):
                    mm(
                        ps[:],
                        sc_bf[:, dc : dc + 1],
                        wa[:, dc, j * 512 : (j + 1) * 512],
                        start=(dc == 0),
                        stop=(dc == NDC - 1),
                    )
                v.tensor_copy(arow[:, j * 512 : (j + 1) * 512], ps[:])
            adram = dram_pool.tile([1, 12 * P], F32, name=f"adram{r}")
            dma(out=adram[:], in_=arow[:])
            acol = pool.tile([P, 12], F32, name=f"acol{r}")
            dma(
                out=acol[:],
                in_=adram[0, :].rearrange("(c p) -> p c", p=P),
            )
            v.tensor_add(acol[:], acol[:], bc[:])
            v.tensor_copy(sh_col[:], acol[:, 0:6])
            v.tensor_scalar_add(sp_col[:], acol[:, 6:12], 1.0)
            # gate, row layout then broadcast across partitions
            g_row = pool.tile([1, D], F32, name=f"grow{r}")
            for j, (n0, n1) in enumerate([(0, 512), (512, 768)]):
                ps = psA2.tile([1, n1 - n0], F32, tag="psg")
                for dc in range(NDC):
                    mm(
                        ps[:],
                        sc_bf[:, dc : dc + 1],
                        wa[:, dc, 2 * D + n0 : 2 * D + n1],
                        start=(dc == 0),
                        stop=(dc == NDC - 1),
                    )
                v.tensor_add(g_row[:, n0:n1], ps[:], bg[:, n0:n1])
            nc.gpsimd.partition_broadcast(g_b[:], g_row[:])

    # ---------------- phase B: LN1 + modulation + QKV projection ----------
    big = ctx.enter_context(tc.tile_pool(name="big", bufs=1))
    x_loc = big.tile([P, NQC, D], F32)  # local rows for the residual
    x2_loc = [big.tile([P, D], F32, name=f"x2_loc{q}") for q in range(NQC)]
    catT_all = big.tile([P, NDC, LQ], BF16)  # attention output, transposed
    xn2T = big.tile([P, NDC, LQ], BF16)
    s_attn = ctx.enter_context(ExitStack())
    attn_pool = s_attn.enter_context(tc.tile_pool(name="attn", bufs=1))
    xn1T = [
        attn_pool.tile([P, NDC, 512], BF16, name=f"xn1T_{i}")
        for i in range(NLC)
    ]  # modulated LN1 output, transposed, one strip per 512 rows
    v_all = attn_pool.tile([P, NKC, H * (HD + 1)], BF16)  # V with ones column
    qT_all = attn_pool.tile([P, NHP, LQ], BF16)
    wq_kq = attn_pool.tile([P, NDC, 2 * D], BF16)  # Q and K columns of w_qkv
    bq_col = attn_pool.tile([P, 18], F32)

    dma(out=wq_kq[:], in_=t["w_qkv"].rearrange("(c p) m -> p c m", p=P)[:, :, 0 : 2 * D])
    dma(out=bq_col[:], in_=t["b_qkv_col"][:])
    dma(
        out=x_loc[:],
        in_=t["x"][0:LQ, :].rearrange("(n p) d -> p n d", p=P),
    )
    # ones column of the augmented V (gives the softmax denominator)
    v.memset(v_all.rearrange("p k (h e) -> p k h e", e=HD + 1)[:, :, :, HD : HD + 1], 1.0)

    with ExitStack() as phB:
        wqv_pool = phB.enter_context(tc.tile_pool(name="wqv", bufs=1))
        xload = phB.enter_context(tc.tile_pool(name="xload", bufs=3))
        spool = phB.enter_context(tc.tile_pool(name="spool", bufs=3))
        nxpool = phB.enter_context(tc.tile_pool(name="nxpool", bufs=2))
        psB1 = phB.enter_context(tc.tile_pool(name="psB1", bufs=2, space="PSUM"))
        psB2 = phB.enter_context(tc.tile_pool(name="psB2", bufs=2, space="PSUM"))
        psBt = phB.enter_context(tc.tile_pool(name="psBt", bufs=2, space="PSUM"))

        wq_v = wqv_pool.tile([P, NDC, D], BF16)
        dma(out=wq_v[:], in_=t["w_qkv"].rearrange("(c p) m -> p c m", p=P)[:, :, 2 * D : 3 * D])
        bv_b = wqv_pool.tile([P, D], F32)
        dma(out=bv_b[:], in_=t["b_v_b"][:])

        # LN1 + AdaLN modulation, written transposed into xn1T
        x_r = t["x"].rearrange("(n p) d -> n p d", p=P)
        for i in range(NKC):
            xt = xload.tile([P, D], F32)
            dma(out=xt[:], in_=x_r[i])
            stats = spool.tile([P, 2, 6], F32)
            for g in range(2):
                v.bn_stats(stats[:, g, :], xt[:, g * 384 : (g + 1) * 384])
            mv = spool.tile([P, 2], F32)
            v.bn_aggr(mv[:], stats[:])
            sq = spool.tile([P, 1], F32)
            act(sq[:], mv[:, 1:2], AF.Sqrt, bias=eps_t[:, 0:1])
            rstd = spool.tile([P, 1], F32)
            v.reciprocal_approx_fast(rstd[:], sq[:])
            nx = nxpool.tile([P, D], BF16)
            v.tensor_scalar(
                nx[:], xt[:], mv[:, 0:1], rstd[:], op0=OP.subtract, op1=OP.mult
            )
            for dc in range(NDC):
                pt = psBt.tile([P, P], BF16)
                nc.tensor.transpose(pt[:], nx[:, dc * P : (dc + 1) * P], identity[:])
                act(
                    xn1T[i // 4][:, dc, (i % 4) * P : (i % 4 + 1) * P],
                    pt[:],
                    AF.Identity,
                    bias=sh1_col[:, dc : dc + 1],
                    scale=sp1_col[:, dc : dc + 1],
                )
        # V for all heads, natural layout (+ bias), ones column interleaved
        v4 = v_all.rearrange("p k (h e) -> p k h e", e=HD + 1)
        bv3 = bv_b.rearrange("p (h e) -> p h e", e=HD)
        for kc in range(NKC):
            ps_v = psB2.tile([P, D], F32)
            for dc in range(NDC):
                lhs = xn1T[kc // 4][:, dc, (kc % 4) * P : (kc % 4 + 1) * P]
                mm(ps_v[:, 0:512], lhs, wq_v[:, dc, 0:512],
                   start=(dc == 0), stop=(dc == NDC - 1))
                mm(ps_v[:, 512:768], lhs, wq_v[:, dc, 512:768],
                   start=(dc == 0), stop=(dc == NDC - 1))
            v.tensor_tensor(
                v4[:, kc, :, 0:HD],
                ps_v.rearrange("p (h e) -> p h e", e=HD),
                bv3[:],
                op=OP.add,
            )
        # Q^T (local rows only -- they are the first l-chunk)
        for hp in range(NHP):
            ps_q = psB1.tile([P, 512], F32, tag="mm512")
            for dc in range(NDC):
                mm(
                    ps_q[:],
                    wq_kq[:, dc, hp * P : (hp + 1) * P],
                    xn1T[0][:, dc, :],
                    start=(dc == 0),
                    stop=(dc == NDC - 1),
                )
            v.tensor_scalar_add(qT_all[:, hp, :], ps_q[:], bq_col[:, hp : hp + 1])

    # ------- phase C: merged K-projection + attention pipeline -------------
    with ExitStack() as phC:
        kv_pool = phC.enter_context(tc.tile_pool(name="kvp", bufs=2))
        pt_pool = phC.enter_context(tc.tile_pool(name="ptp", bufs=3))
        tiny = phC.enter_context(tc.tile_pool(name="tiny", bufs=3))
        rzb_pool = phC.enter_context(tc.tile_pool(name="rzb", bufs=3))
        psS = phC.enter_context(tc.tile_pool(name="psS", bufs=2, space="PSUM"))
        psO = phC.enter_context(tc.tile_pool(name="psO", bufs=2, space="PSUM"))
        psK = phC.enter_context(tc.tile_pool(name="psK", bufs=2, space="PSUM"))

        for hp in range(NHP):
            # K^T for this head pair (overlaps with attention on hp-1)
            kT = kv_pool.tile([P, L], BF16, tag="kT")
            for lc in range(NLC):
                ps_k = psK.tile([P, 512], F32)
                for dc in range(NDC):
                    mm(
                        ps_k[:],
                        wq_kq[:, dc, D + hp * P : D + (hp + 1) * P],
                        xn1T[lc][:, dc, :],
                        start=(dc == 0),
                        stop=(dc == NDC - 1),
                    )
                v.tensor_scalar_add(
                    kT[:, lc * 512 : (lc + 1) * 512],
                    ps_k[:],
                    bq_col[:, 6 + hp : 7 + hp],
                )
            for dlt in range(2):
                h, off = 2 * hp + dlt, dlt * HD
                ps_o = psO.tile([HD + 1, 512], F32)
                for kc2 in range(NKC // 2):
                    ps_s = psS.tile([P, 1024], F32)
                    for j in range(2):
                        kc = 2 * kc2 + j
                        mm(
                            ps_s[:, j * 512 : (j + 1) * 512],
                            kT[off : off + HD, kc * P : (kc + 1) * P],
                            qT_all[off : off + HD, hp, :],
                            start=True,
                            stop=True,
                        )
                    ptile = pt_pool.tile([P, 1024], BF16)
                    act(ptile[:], ps_s[:], AF.Exp, scale=0.125)
                    for j in range(2):
                        kc = 2 * kc2 + j
                        mm(
                            ps_o[:],
                            v_all[:, kc, h * (HD + 1) : (h + 1) * (HD + 1)],
                            ptile[:, j * 512 : (j + 1) * 512],
                            start=(kc == 0),
                            stop=(kc == NKC - 1),
                        )
                # normalize columns by the ones-row (softmax denominator)
                zrow = tiny.tile([1, 512], F32)
                v.tensor_copy(zrow[:], ps_o[HD : HD + 1, :])
                rz = tiny.tile([1, 512], F32, tag="rz")
                v.reciprocal_approx_fast(rz[:], zrow[:])
                rz_b = rzb_pool.tile([P, 512], F32)
                nc.gpsimd.partition_broadcast(rz_b[:], rz[:])
                v.tensor_tensor(
                    catT_all[off : off + HD, hp, :],
                    ps_o[0:HD, :],
                    rz_b[0:HD, :],
                    op=OP.mult,
                )

    s_attn.close()  # free K/V/Q space before the FFN weights land

    # -------- phase D+E: out-projection, residual, LN2 (per-qc fused) ------
    with ExitStack() as phD:
        pool = phD.enter_context(tc.tile_pool(name="phD", bufs=2))
        wpool = phD.enter_context(tc.tile_pool(name="wao", bufs=1))
        spool = phD.enter_context(tc.tile_pool(name="spoolE", bufs=3))
        nxpool = phD.enter_context(tc.tile_pool(name="nxE", bufs=2))
        psD1 = phD.enter_context(tc.tile_pool(name="psD1", bufs=2, space="PSUM"))
        psD2 = phD.enter_context(tc.tile_pool(name="psD2", bufs=2, space="PSUM"))
        psEt = phD.enter_context(tc.tile_pool(name="psEt", bufs=2, space="PSUM"))

        wao_sb = wpool.tile([P, NDC, D], BF16)
        dma(out=wao_sb[:], in_=t["w_attn_out"].rearrange("(c p) m -> p c m", p=P))
        ba_sb = wpool.tile([P, D], F32)
        dma(out=ba_sb[:], in_=t["b_attn_b"][:])

        for qc in range(NQC):
            ps1 = psD1.tile([P, 512], F32)
            ps2 = psD2.tile([P, 256], F32)
            for cc in range(NDC):
                lhs = catT_all[:, cc, qc * P : (qc + 1) * P]
                mm(ps1[:], lhs, wao_sb[:, cc, 0:512],
                   start=(cc == 0), stop=(cc == NDC - 1))
                mm(ps2[:], lhs, wao_sb[:, cc, 512:768],
                   start=(cc == 0), stop=(cc == NDC - 1))
            yb = pool.tile([P, D], F32)
            v.tensor_add(yb[:, 0:512], ps1[:], ba_sb[:, 0:512])
            v.tensor_add(yb[:, 512:768], ps2[:], ba_sb[:, 512:768])
            yg = pool.tile([P, D], F32)
            v.tensor_tensor(yg[:], yb[:], g1_b[:], op=OP.mult)
            v.tensor_add(x2_loc[qc][:], yg[:], x_loc[:, qc, :])
            # LN2 for this q-chunk immediately (hides under next out-proj)
            xq = x2_loc[qc][:]
            stats = spool.tile([P, 2, 6], F32)
            for g in range(2):
                v.bn_stats(stats[:, g, :], xq[:, g * 384 : (g + 1) * 384])
            mv = spool.tile([P, 2], F32)
            v.bn_aggr(mv[:], stats[:])
            sq = spool.tile([P, 1], F32)
            act(sq[:], mv[:, 1:2], AF.Sqrt, bias=eps_t[:, 0:1])
            rstd = spool.tile([P, 1], F32)
            v.reciprocal_approx_fast(rstd[:], sq[:])
            nx = nxpool.tile([P, D], BF16)
            v.tensor_scalar(
                nx[:], xq, mv[:, 0:1], rstd[:], op0=OP.subtract, op1=OP.mult
            )
            for dc in range(NDC):
                pt = psEt.tile([P, P], BF16)
                nc.tensor.transpose(pt[:], nx[:, dc * P : (dc + 1) * P], identity[:])
                act(
                    xn2T[:, dc, qc * P : (qc + 1) * P],
                    pt[:],
                    AF.Identity,
                    bias=sh2_col[:, dc : dc + 1],
                    scale=sp2_col[:, dc : dc + 1],
                )

    # ---------------- phase F: FFN + gate + residual -> output -------------
    with ExitStack() as phF:
        wpool = phF.enter_context(tc.tile_pool(name="wffn", bufs=1))
        hpool = phF.enter_context(tc.tile_pool(name="hT", bufs=1))
        pool = phF.enter_context(tc.tile_pool(name="phF", bufs=2))
        psF1 = phF.enter_context(tc.tile_pool(name="psF1", bufs=3, space="PSUM"))
        psF2 = phF.enter_context(tc.tile_pool(name="psF2", bufs=2, space="PSUM"))

        wf1_sb = wpool.tile([P, NDC, DM], BF16)
        dma(out=wf1_sb[:], in_=t["w_ffn1"].rearrange("(c p) m -> p c m", p=P))
        bf1_col = wpool.tile([P, NMC], F32)
        dma(out=bf1_col[:], in_=t["b_ffn1_col"][:])
        wf2_sb = wpool.tile([P, NMC, D], BF16)
        dma(out=wf2_sb[:], in_=t["w_ffn2"].rearrange("(c p) m -> p c m", p=P))
        bf2_b = wpool.tile([P, D], F32)
        dma(out=bf2_b[:], in_=t["b_ffn2_b"][:])

        hT_all = hpool.tile([P, NMC, LQ], BF16)
        for mc in range(NMC):
            ps_h = psF1.tile([P, 512], F32, tag="mm512")
            for dc in range(NDC):
                mm(
                    ps_h[:],
                    wf1_sb[:, dc, mc * P : (mc + 1) * P],
                    xn2T[:, dc, :],
                    start=(dc == 0),
                    stop=(dc == NDC - 1),
                )
            act(hT_all[:, mc, :], ps_h[:], AF.Gelu, bias=bf1_col[:, mc : mc + 1])

        out_r = t["out"].rearrange("(n p) d -> n p d", p=P)
        for qc in range(NQC):
            ps1 = psF1.tile([P, 512], F32, tag="mm512")
            ps2 = psF2.tile([P, 256], F32)
            for mc in range(NMC):
                lhs = hT_all[:, mc, qc * P : (qc + 1) * P]
                mm(ps1[:], lhs, wf2_sb[:, mc, 0:512],
                   start=(mc == 0), stop=(mc == NMC - 1))
                mm(ps2[:], lhs, wf2_sb[:, mc, 512:768],
                   start=(mc == 0), stop=(mc == NMC - 1))
            y2 = pool.tile([P, D], F32)
            v.tensor_add(y2[:, 0:512], ps1[:], bf2_b[:, 0:512])
            v.tensor_add(y2[:, 512:768], ps2[:], bf2_b[:, 512:768])
            yg = pool.tile([P, D], F32)
            v.tensor_tensor(yg[:], y2[:], g2_b[:], op=OP.mult)
            ot = pool.tile([P, D], F32)
            v.tensor_add(ot[:], yg[:], x2_loc[qc][:])
            dma(out=out_r[qc], in_=ot[:])


def build_nc():
    nc = bacc.Bacc(None, target_bir_lowering=False, debug=False)
    t = _declare_params(nc)
    with tile.TileContext(nc) as tc:
        with ExitStack() as ctx:
            _build_body(nc, tc, ctx, t)
    nc.compile()
    return nc


_cache = {}


def _prep_in_maps(inputs):
    bf = lambda a: np.ascontiguousarray(np.asarray(a, np.float32)).astype(
        ml_dtypes.bfloat16
    )
    f32 = lambda a: np.ascontiguousarray(np.asarray(a, np.float32))
    x = f32(inputs["x"]).reshape(L, D)
    cond = f32(inputs["cond"]).reshape(D)
    b_adaln1 = f32(inputs["b_adaln1"]).reshape(3 * D)
    b_adaln2 = f32(inputs["b_adaln2"]).reshape(3 * D)
    b_qkv = f32(inputs["b_qkv"]).reshape(3 * D)
    common = {
        "cond_t": np.ascontiguousarray(cond.reshape(NDC, P).T),
        "w_adaln1": bf(inputs["w_adaln1"]),
        "w_adaln2": bf(inputs["w_adaln2"]),
        "b_adaln1_col": np.ascontiguousarray(b_adaln1[: 12 * P].reshape(12, P).T),
        "b_adaln2_col": np.ascontiguousarray(b_adaln2[: 12 * P].reshape(12, P).T),
        "b_adaln1_gate": np.ascontiguousarray(b_adaln1[2 * D :][None]),
        "b_adaln2_gate": np.ascontiguousarray(b_adaln2[2 * D :][None]),
        "w_qkv": bf(inputs["w_qkv"]),
        "b_qkv_col": np.ascontiguousarray(b_qkv.reshape(18, P).T),
        "b_v_b": np.ascontiguousarray(np.broadcast_to(b_qkv[2 * D :], (P, D))),
        "w_attn_out": bf(inputs["w_attn_out"]),
        "b_attn_b": np.ascontiguousarray(
            np.broadcast_to(f32(inputs["b_attn_out"]).reshape(D), (P, D))
        ),
        "w_ffn1": bf(inputs["w_ffn1"]),
        "b_ffn1_col": np.ascontiguousarray(
            f32(inputs["b_ffn1"]).reshape(NMC, P).T
        ),
        "w_ffn2": bf(inputs["w_ffn2"]),
        "b_ffn2_b": np.ascontiguousarray(
            np.broadcast_to(f32(inputs["b_ffn2"]).reshape(D), (P, D))
        ),
    }
    in_maps = []
    for c in range(NCORES):
        m = dict(common)
        m["x"] = np.ascontiguousarray(np.roll(x, -c * LQ, axis=0))
        in_maps.append(m)
    return in_maps


def kernel(**inputs):
    if "nc" not in _cache:
        _cache["nc"] = build_nc()
    nc = _cache["nc"]
    in_maps = _prep_in_maps(inputs)
    res = run_bass_kernel_spmd(nc, in_maps, list(range(NCORES)))
    out = np.concatenate([res.results[c]["out"] for c in range(NCORES)], axis=0)
    return out.reshape(1, L, D).astype(np.float32)


if __name__ == "__main__":
    rng = np.random.default_rng(0)
    fake = {
        "x": rng.standard_normal((1, L, D), dtype=np.float32),
        "cond": rng.standard_normal((1, D), dtype=np.float32),
        "w_adaln1": rng.standard_normal((D, 3 * D), dtype=np.float32) * 0.02,
        "b_adaln1": np.zeros(3 * D, np.float32),
        "w_qkv": rng.standard_normal((D, 3 * D), dtype=np.float32) * D**-0.5,
        "b_qkv": np.zeros(3 * D, np.float32),
        "w_attn_out": rng.standard_normal((D, D), dtype=np.float32) * D**-0.5,
        "b_attn_out": np.zeros(D, np.float32),
        "w_adaln2": rng.standard_normal((D, 3 * D), dtype=np.float32) * 0.02,
        "b_adaln2": np.zeros(3 * D, np.float32),
        "w_ffn1": rng.standard_normal((D, DM), dtype=np.float32) * D**-0.5,
        "b_ffn1": np.zeros(DM, np.float32),
        "w_ffn2": rng.standard_normal((DM, D), dtype=np.float32) * DM**-0.5,
        "b_ffn2": np.zeros(D, np.float32),
    }
    out = kernel(**fake)
    print("out", out.shape, out.dtype, np.abs(out).max())
